# revision 13
# baseline (speedup 1.0000x reference)
# DILATE loss (soft-DTW shape + temporal) Trainium2 Bass kernel.
#
# Math: with gamma=1, soft-DTW forward and its gradient are computed in the
# exp-domain where both passes are pure multiply-add wavefront recurrences:
#   forward : G[i,j] = exp(-D[i,j]) * (G[i-1,j-1] + G[i-1,j] + G[i,j-1])
#   backward: F[i,j] = exp(-D[i,j]) * (F[i+1,j+1] + F[i+1,j] + F[i,j+1])
#   E[i,j]  = G[i,j] * (F[i+1,j] + F[i,j+1] + F[i+1,j+1])
# with per-diagonal-step per-batch renormalization (tracked in log domain).
#
# Layout per core (32 batch elements): partitions p = 32*c + b, where c is a
# quarter-chunk of the anti-diagonal slot axis and b the batch element. The
# exp(-D) matrix rows live SBUF-resident per partition with a per-chunk baked
# byte shift so each diagonal step's operand is a single affine AP
# (offset d, stride ROWPITCH-1).
import numpy as np

N = 336
B = 256
V = 7
NCORES = 8
BP = B // NCORES          # batch per core
SL = N + 1                # diagonal slot count (i = 0..N)
QS = (SL + 3) // 4        # slots per chunk (85)
RP = N + 2                # row pitch in floats (j = 0..N+1; col 0 and N+1 zero)
REGF = QS * (RP + 3) + RP  # per-partition floats for expD region
ND = 2 * N                # last diagonal index
ALPHA = 0.5

_PROGRAM = None


def _split_multi_waits(nc):
    # this toolchain encodes at most one sem-wait per instruction; Tile can
    # emit several -- split extras onto NoOps placed just before
    from concourse import mybir
    for blk in nc.bb_map.values():
        bb = blk.bb
        newlist = []
        changed = False
        for inst in bb.instructions:
            si = getattr(inst, 'sync_info', None)
            if si is not None and si.on_wait and len(si.on_wait) > 1:
                waits = list(si.on_wait)
                for w in waits[:-1]:
                    nop = mybir.InstNoOp(name=nc.get_next_instruction_name(),
                                         ins=[], outs=[])
                    nop.engine = inst.engine
                    nop.sync_info = mybir.SyncInfo(on_wait=[w], on_update=[])
                    nc.register_instruction(nop, overwrite=True)
                    newlist.append(nop)
                si.on_wait = [waits[-1]]
                changed = True
            newlist.append(inst)
        if changed:
            bb.instructions = newlist
    return nc


def _build_program():
    import concourse.bass as bass
    import concourse.tile as tile
    from concourse import mybir

    f32 = mybir.dt.float32
    AF = mybir.ActivationFunctionType
    OP = mybir.AluOpType
    AX = mybir.AxisListType

    nc = bass.Bass()
    ABbar = nc.declare_dram_parameter("ABbar", [9, BP, 4 * QS + N], f32,
                                      isOutput=False)
    Omg = nc.declare_dram_parameter("Omg", [ND - 1, 128, QS], f32, isOutput=False)
    Out = nc.declare_dram_parameter("Out", [2, BP], f32, isOutput=True)
    gspill = nc.dram_tensor("gspill", [ND + 1, 128, QS], f32)

    with tile.TileContext(nc) as tc:
        with (
            tc.tile_pool(name="big", bufs=1) as big,
            tc.tile_pool(name="gtiles", bufs=6) as gpool,
            tc.tile_pool(name="raw", bufs=3) as rawpool,
            tc.tile_pool(name="t1p", bufs=3) as t1pool,
            tc.tile_pool(name="tiny", bufs=8) as tiny,
            tc.tile_pool(name="stage", bufs=4) as stage,
            tc.tile_pool(name="psum", bufs=8, space="PSUM") as pp,
            tc.tile_pool(name="stream", bufs=6) as stream,
        ):
            expd = big.tile([128, REGF], f32)
            mh = big.tile([128, ND + 2], f32)       # m history (fwd)
            imh = big.tile([128, ND + 2], f32)      # 1/m history (fwd)
            slog = big.tile([128, ND + 2], f32)     # cumsum log m
            zcol = big.tile([128, ND + 2], f32)     # zeros for scan
            imfh = big.tile([128, ND + 3], f32)     # 1/mF history (bwd)

            # zero the expD region (guard cols/rows and pads must be exactly 0)
            nc.vector.memset(expd[:], 0.0)
            nc.gpsimd.memset(mh[:], 1.0)
            nc.gpsimd.memset(imh[:], 1.0)
            nc.gpsimd.memset(imfh[:], 1.0)
            nc.vector.memset(zcol[:], 0.0)

            tc.strict_bb_all_engine_barrier()

            # ---- precompute exp(-D) into the chunked skewed layout ----
            for b in range(BP):
                abt = stage.tile([9, 4 * QS + N], f32, tag="abt")
                nc.sync.dma_start(abt[:], ABbar[:, b])
                for c in range(4):
                    ps = pp.tile([QS, N], f32, tag="ps")
                    lhsT = abt[:, c * QS: (c + 1) * QS]
                    rhs = abt[:, 4 * QS:]
                    nc.tensor.matmul(ps[:], lhsT, rhs, start=True, stop=True)
                    stg = stage.tile([QS, N], f32, tag="stg")
                    nc.scalar.activation(stg[:], ps[:], AF.Exp)
                    # rows s of chunk c for batch b -> partition 32c+b,
                    # float offset QS*c + s*RP + 1
                    p = 32 * c + b
                    dst = expd[p: p + 1, QS * c + 1: QS * c + 1 + QS * RP]
                    dst = dst.rearrange("p (s j) -> p s j", j=RP)[:, :, 0:N]
                    nc.sync.dma_start(dst, stg[:])

            def ed_ap(d):
                v = expd[:, d: d + QS * (RP - 1)]
                return v.rearrange("p (s j) -> p s j", j=RP - 1)[:, :, 0:1]

            # ---- forward ----
            G = {}
            g0 = gpool.tile([128, QS + 2], f32, tag="g")
            nc.vector.memset(g0[:], 0.0)
            nc.vector.memset(g0[0:32, 1:2], 1.0)    # G[0,0] = 1 (chunk0 slot0)
            g1 = gpool.tile([128, QS + 2], f32, tag="g")
            nc.vector.memset(g1[:], 0.0)
            G[0], G[1] = g0, g1

            for d in range(2, ND + 1):
                Gp, Gpp = G[d - 1], G[d - 2]
                t1 = t1pool.tile([128, QS], f32, tag="t1")
                nc.vector.tensor_tensor(t1[:], Gp[:, 1: QS + 1], Gp[:, 0: QS], OP.add)
                nc.vector.scalar_tensor_tensor(
                    t1[:], Gpp[:, 0: QS], imh[:, d - 1: d], t1[:], OP.mult, OP.add)
                raw = rawpool.tile([128, QS], f32, tag="raw")
                nc.vector.tensor_tensor(
                    raw[:], t1[:], ed_ap(d).rearrange("p s j -> p (s j)"), OP.mult)
                # per-batch max across the 4 chunks (stack + reduce;
                # cross-base tensor_tensor is rejected by the BIR verifier)
                q = tiny.tile([128, 1], f32, tag="q")
                nc.vector.tensor_reduce(q[:], raw[:], AX.X, OP.max)
                qs = tiny.tile([128, 4], f32, tag="qs")
                for qq in range(4):
                    nc.scalar.copy(qs[0:32, qq: qq + 1], q[32 * qq: 32 * qq + 32, :])
                qb = tiny.tile([128, 1], f32, tag="qb")
                nc.vector.tensor_reduce(qb[0:32, :], qs[0:32, :], AX.X, OP.max)
                qg = tiny.tile([128, 1], f32, tag="qg")
                nc.vector.tensor_scalar_max(qg[0:32, :], qb[0:32, :], 1e-30)
                for qq in range(4):
                    nc.scalar.copy(mh[32 * qq: 32 * qq + 32, d: d + 1], qg[0:32, :])
                nc.vector.reciprocal(imh[:, d: d + 1], mh[:, d: d + 1])
                g = gpool.tile([128, QS + 2], f32, tag="g")
                nc.vector.tensor_scalar_mul(g[:, 1: QS + 1], raw[:], imh[:, d: d + 1])
                # halo: chunk c slot -1 <- chunk c-1 slot QS-1
                nc.vector.memset(g[0:32, 0:1], 0.0)
                nc.scalar.copy(g[32:64, 0:1], g[0:32, QS: QS + 1])
                nc.scalar.copy(g[64:96, 0:1], g[32:64, QS: QS + 1])
                nc.scalar.copy(g[96:128, 0:1], g[64:96, QS: QS + 1])
                nc.sync.dma_start(gspill[d], g[:, 1: QS + 1])
                G[d] = g
                if d - 3 in G and d - 3 >= 2:
                    del G[d - 3]

            # SlogG = cumsum(log m)
            lg = big.tile([128, ND + 2], f32)
            nc.scalar.activation(lg[:], mh[:], AF.Ln)
            nc.vector.tensor_tensor_scan(
                slog[:], lg[:], zcol[:], 0.0, OP.add, OP.add)

            # loss_shape per batch: R_NN = -(log G'[ND][slot N] + SlogG[ND])
            c_nn = (N // QS)
            k_nn = N - c_nn * QS + 1
            pb_nn = 32 * c_nn
            r1 = tiny.tile([128, 1], f32, tag="r1")
            nc.scalar.activation(
                r1[pb_nn: pb_nn + 32, :], G[ND][pb_nn: pb_nn + 32, k_nn: k_nn + 1],
                AF.Ln)
            r2 = tiny.tile([128, 1], f32, tag="r2")
            nc.vector.tensor_tensor(
                r2[pb_nn: pb_nn + 32, :], r1[pb_nn: pb_nn + 32, :],
                slog[pb_nn: pb_nn + 32, ND: ND + 1], OP.add)
            nc.vector.tensor_scalar_mul(r1[pb_nn: pb_nn + 32, :],
                                        r2[pb_nn: pb_nn + 32, :], -1.0)
            nc.sync.dma_start(Out[0], r1[pb_nn: pb_nn + 32, 0])

            # ---- backward ----
            F = {}
            fs = gpool.tile([128, QS + 2], f32, tag="g")
            nc.vector.memset(fs[:], 0.0)
            rg = tiny.tile([128, 1], f32, tag="rg")
            nc.vector.reciprocal(rg[pb_nn: pb_nn + 32, :],
                                 G[ND][pb_nn: pb_nn + 32, k_nn: k_nn + 1])
            nc.vector.tensor_copy(fs[pb_nn: pb_nn + 32, k_nn: k_nn + 1],
                                  rg[pb_nn: pb_nn + 32, :])
            f2 = gpool.tile([128, QS + 2], f32, tag="g")
            nc.vector.memset(f2[:], 0.0)
            F[ND + 1], F[ND + 2] = fs, f2

            tl = big.tile([128, 2], f32)            # TlogF (alternating cols)
            nc.vector.tensor_scalar_mul(tl[:, 0:1], slog[:, ND: ND + 1], -1.0)
            ta = big.tile([128, 2], f32)            # tacc (alternating cols)
            nc.gpsimd.memset(ta[:], 0.0)

            for step, d in enumerate(range(ND, 1, -1)):
                Fp, Fpp = F[d + 1], F[d + 2]
                cur, nxt = step % 2, (step + 1) % 2
                t1 = t1pool.tile([128, QS], f32, tag="t1")
                nc.vector.tensor_tensor(
                    t1[:], Fp[:, 1: QS + 1], Fp[:, 2: QS + 2], OP.add)
                nc.vector.scalar_tensor_tensor(
                    t1[:], Fpp[:, 2: QS + 2], imfh[:, d + 1: d + 2], t1[:],
                    OP.mult, OP.add)
                raw = rawpool.tile([128, QS], f32, tag="raw")
                nc.vector.tensor_tensor(
                    raw[:], t1[:], ed_ap(d).rearrange("p s j -> p (s j)"), OP.mult)
                # E contribution
                gl = stream.tile([128, QS], f32, tag="gl")
                nc.sync.dma_start(gl[:], gspill[d])
                # E = eps * t1 * G'; eps = exp(SlogG+TlogF) can exceed fp32
                # range, so fold sqrt(eps) into each factor instead
                eps = tiny.tile([128, 1], f32, tag="eps")
                nc.vector.tensor_tensor(
                    eps[:], slog[:, d: d + 1], tl[:, cur: cur + 1], OP.add)
                nc.scalar.activation(eps[:], eps[:], AF.Exp, scale=0.5)
                t1s = rawpool.tile([128, QS], f32, tag="t1s")
                nc.vector.tensor_scalar_mul(t1s[:], t1[:], eps[:])
                ee = rawpool.tile([128, QS], f32, tag="ee")
                nc.vector.scalar_tensor_tensor(
                    ee[:], gl[:], eps[:], t1s[:], OP.mult, OP.mult)
                om = stream.tile([128, QS], f32, tag="om")
                nc.sync.dma_start(om[:], Omg[d - 2])
                ew = rawpool.tile([128, QS], f32, tag="ew")
                red = tiny.tile([128, 1], f32, tag="red")
                nc.vector.scalar_tensor_tensor(
                    ew[:], ee[:], 1.0, om[:], OP.mult, OP.mult, accum_out=red[:])
                nc.vector.scalar_tensor_tensor(
                    ta[:, nxt: nxt + 1], red[:], 1.0, ta[:, cur: cur + 1],
                    OP.mult, OP.add)
                # renorm F
                q = tiny.tile([128, 1], f32, tag="q")
                nc.vector.tensor_reduce(q[:], raw[:], AX.X, OP.max)
                qs = tiny.tile([128, 4], f32, tag="qs")
                for qq in range(4):
                    nc.scalar.copy(qs[0:32, qq: qq + 1], q[32 * qq: 32 * qq + 32, :])
                qb = tiny.tile([128, 1], f32, tag="qb")
                nc.vector.tensor_reduce(qb[0:32, :], qs[0:32, :], AX.X, OP.max)
                qg = tiny.tile([128, 1], f32, tag="qg")
                nc.vector.tensor_scalar_max(qg[0:32, :], qb[0:32, :], 1e-30)
                mfc = tiny.tile([128, 1], f32, tag="mfc")
                for qq in range(4):
                    nc.scalar.copy(mfc[32 * qq: 32 * qq + 32, :], qg[0:32, :])
                nc.vector.reciprocal(imfh[:, d: d + 1], mfc[:])
                f = gpool.tile([128, QS + 2], f32, tag="g")
                nc.vector.tensor_scalar_mul(f[:, 1: QS + 1], raw[:],
                                            imfh[:, d: d + 1])
                # right halo: chunk c slot QS <- chunk c+1 slot 0
                nc.vector.memset(f[96:128, QS + 1: QS + 2], 0.0)
                nc.scalar.copy(f[64:96, QS + 1: QS + 2], f[96:128, 1:2])
                nc.scalar.copy(f[32:64, QS + 1: QS + 2], f[64:96, 1:2])
                nc.scalar.copy(f[0:32, QS + 1: QS + 2], f[32:64, 1:2])
                # TlogF += log mF
                lgf = tiny.tile([128, 1], f32, tag="lgf")
                nc.scalar.activation(lgf[:], mfc[:], AF.Ln)
                nc.vector.tensor_tensor(
                    tl[:, nxt: nxt + 1], tl[:, cur: cur + 1], lgf[:], OP.add)
                F[d] = f
                if d == ND:
                    # the seed tile is virtual: it must not be re-read as the
                    # lag-2 (diagonal-child) operand in the next step
                    nc.vector.memset(fs[:], 0.0)
                if d + 3 in F:
                    del F[d + 3]

            # tacc: sum the 4 chunks, write out
            last = (ND - 1) % 2
            s1 = tiny.tile([128, 4], f32, tag="s1")
            for qq in range(4):
                nc.scalar.copy(s1[0:32, qq: qq + 1],
                               ta[32 * qq: 32 * qq + 32, last: last + 1])
            s2 = tiny.tile([128, 1], f32, tag="s2")
            nc.vector.tensor_reduce(s2[0:32, :], s1[0:32, :], AX.X, OP.add)
            nc.sync.dma_start(Out[1], s2[0:32, 0])
    return _split_multi_waits(nc)


def _host_inputs(outputs, targets):
    f32 = np.float32
    T = targets.astype(f32)
    O = outputs.astype(f32)
    tn = (T * T).sum(-1)
    on = (O * O).sum(-1)
    s2 = np.float32(np.sqrt(2.0))
    in_maps = []
    # omega diag tiles, shared across cores: om[d-2][32c+b, s] = (2*(QS*c+s)-d)^2
    om = np.zeros((ND - 1, 128, QS), f32)
    cc = (np.arange(128) // 32)
    ss = np.arange(QS)
    ii = (QS * cc[:, None] + ss[None, :]).astype(np.int64)  # (128, QS)
    for d in range(2, ND + 1):
        w = ((2 * ii - d).astype(f32)) ** 2
        # zero out slots outside the valid diagonal range: leak cells there
        # carry garbage that the eps scale factor would amplify
        valid = (ii >= max(1, d - N)) & (ii <= min(N, d - 1))
        om[d - 2] = np.where(valid, w, np.float32(0.0))
    for k in range(NCORES):
        sl = slice(k * BP, (k + 1) * BP)
        Tb, Ob = T[sl], O[sl]
        AB = np.zeros((9, BP, 4 * QS + N), f32)
        AB[0:7, :, 1: N + 1] = (s2 * Tb).transpose(2, 0, 1)
        AB[7, :, 1: N + 1] = 1.0
        AB[8, :, 1: N + 1] = tn[sl]
        inv = [0] + list(range(N + 1, 4 * QS))
        AB[8, :, inv] = np.float32(200.0)
        AB[0:7, :, 4 * QS:] = (s2 * Ob).transpose(2, 0, 1)
        AB[7, :, 4 * QS:] = -on[sl]
        AB[8, :, 4 * QS:] = -1.0
        in_maps.append({"ABbar": AB, "Omg": om})
    return in_maps


def _tacc_f64(outputs, targets):
    # fp64 soft-DTW grad tacc for a small set of batches (host fallback for
    # batches whose fwd/bwd scale gap exceeds the fp32 exp range)
    dt = np.float64
    Bs, n, _ = outputs.shape
    T = targets.astype(dt)
    O = outputs.astype(dt)
    tn = (T * T).sum(-1)
    on = (O * O).sum(-1)
    D = np.maximum(tn[:, :, None] + on[:, None, :]
                   - 2 * np.einsum('biv,bjv->bij', T, O), 0)
    eD = np.zeros((Bs, n + 1, n + 1), dt)
    eD[:, 1:, 1:] = np.exp(-np.minimum(D, 700))
    # log-domain forward
    BIG = np.inf
    R = np.full((Bs, n + 1, n + 1), BIG, dt)
    R[:, 0, 0] = 0
    for i in range(1, n + 1):
        prev = R[:, i - 1]
        row = np.full((Bs,), BIG, dt)
        out = R[:, i]
        for j in range(1, n + 1):
            a = np.stack([prev[:, j - 1], prev[:, j], out[:, j - 1]])
            m = a.min(0)
            out[:, j] = D[:, i - 1, j - 1] + m - np.log(
                np.exp(np.clip(m - a, -745, 0)).sum(0))
    E = np.zeros((Bs, n + 2, n + 2), dt)
    E[:, n, n] = 1.0
    Rp = np.full((Bs, n + 2, n + 2), np.inf, dt)
    Rp[:, :n + 1, :n + 1] = R
    Dp = np.zeros((Bs, n + 2, n + 2), dt)
    Dp[:, 1:n + 1, 1:n + 1] = D
    for dd in range(2 * n - 1, 1, -1):
        ilo, ihi = max(1, dd - n), min(n, dd - 1)
        ii = np.arange(ilo, ihi + 1)
        jj = dd - ii
        acc = 0
        for (ci, cj) in ((ii + 1, jj), (ii, jj + 1), (ii + 1, jj + 1)):
            w = np.exp(np.clip((Rp[:, ci, cj] - Dp[:, ci, cj]) - Rp[:, ii, jj],
                               -745, 700))
            acc = acc + E[:, ci, cj] * np.where(np.isfinite(w), w, 0)
        E[:, ii, jj] = acc
    idx = np.arange(1, n + 1, dtype=dt)
    omega = (idx[:, None] - idx[None, :]) ** 2
    return (E[:, 1:n + 1, 1:n + 1] * omega[None]).sum(axis=(1, 2))


def kernel(outputs, targets):
    global _PROGRAM
    from concourse.bass_utils import run_bass_kernel_spmd

    outputs = np.asarray(outputs)
    targets = np.asarray(targets)
    if _PROGRAM is None:
        _PROGRAM = _build_program()
    in_maps = _host_inputs(outputs, targets)
    res = run_bass_kernel_spmd(_PROGRAM, in_maps, list(range(NCORES))).results
    r_nn = np.concatenate([np.asarray(r["Out"])[0] for r in res]).astype(np.float64)
    tacc = np.concatenate([np.asarray(r["Out"])[1] for r in res]).astype(np.float64)
    # batches whose scale gap overflowed fp32 produce absurd tacc; redo those
    # few on host in fp64
    bad = np.nonzero(~np.isfinite(tacc) | (np.abs(tacc) > 5e8))[0]
    if len(bad) > 0:
        tacc[bad] = _tacc_f64(outputs[bad], targets[bad])
    loss_shape = r_nn.sum() / B
    loss_temporal = tacc.sum() / (float(N) * N * B * B)
    return np.float32(ALPHA * loss_shape + (1.0 - ALPHA) * loss_temporal)
